# revision 10
# baseline (speedup 1.0000x reference)
"""CODA-Prompt forward kernel for 8 TRN2 NeuronCores (data-parallel over batch).

Reference computation (forward only; stop_gradient is identity):
    K = (task_count + 1) * 10            # active pool slice, all branches
    x_mean[b,d]  = mean_n x[b,n,d]
    aq[b,k]      = (x_mean . (att[k]*nK[k])) / max(||x_mean*att[k]||, eps)
    P_[b,l,d]    = sum_k aq[b,k] * prompt[k,l,d]
    out          = concat([P_, x], axis=1)            # [B, 8+197, 768]

Device kernel per core (B=32 of 256 batches):
    stream x through SBUF once: DMA in -> (copy out to out[:,8:,:]) and
    (token-sum via ones-matmul on PE).  Tiny stage 2/3 computes aq and P_.
The small pool tensors are combined on the host (they are 100x768 scale):
    attnkT[d,k] = att[k,d] * nK[k,d],  attn2T[d,k] = att[k,d]^2,
    prflat[k,:] = prompt[k].reshape(6144)
aq is scale-invariant in x_mean, so the 1/197 mean scaling cancels and the
kernel works with raw token sums.
"""

import numpy as np

TOP_K = 10
LENGTH = 8
EMBED_DIM = 768
N_TOK = 197
B_FULL = 256
N_CORES = 8
B = B_FULL // N_CORES          # 32 batches per core
C = 4                          # batches per DMA chunk
NA = 128                       # tokens in group A
NB = N_TOK - NA                # 69 tokens in group B
H = EMBED_DIM // 384           # 2 psum halves per batch row sum
PF = LENGTH * EMBED_DIM        # 6144 flattened prompt row

_PROGRAMS = {}


def _build_program(K):
    import concourse.bacc as bacc
    import concourse.mybir as mybir
    import concourse.tile as tile
    from concourse.bass import ts

    f32 = mybir.dt.float32
    nc = bacc.Bacc()

    x = nc.dram_tensor("x", [B, N_TOK, EMBED_DIM], f32, kind="ExternalInput")
    prflat = nc.dram_tensor("prflat", [K, PF], f32, kind="ExternalInput")
    attnkT = nc.dram_tensor("attnkT", [EMBED_DIM, K], f32, kind="ExternalInput")
    attn2T = nc.dram_tensor("attn2T", [EMBED_DIM, K], f32, kind="ExternalInput")
    out = nc.dram_tensor("out", [B, LENGTH + N_TOK, EMBED_DIM], f32,
                         kind="ExternalOutput")

    with tile.TileContext(nc) as tc:
        with (
            tc.tile_pool(name="const", bufs=1) as constp,
            tc.tile_pool(name="xa", bufs=3) as xap,
            tc.tile_pool(name="xb", bufs=3) as xbp,
            tc.tile_pool(name="misc", bufs=1) as miscp,
            tc.tile_pool(name="psA", bufs=1, space="PSUM") as psap,
            tc.tile_pool(name="pst", bufs=1, space="PSUM") as pstp,
            tc.tile_pool(name="pp", bufs=3, space="PSUM") as ppp,
            tc.tile_pool(name="scr", bufs=1, space="PSUM") as scrp,
        ):
            # --- constants ---
            ones = constp.tile([128, 1], f32)
            nc.vector.memset(ones, 1.0)
            prflat_sb = constp.tile([K, PF], f32)
            nc.sync.dma_start(out=prflat_sb, in_=prflat[:, :])
            attnkT_sb = constp.tile([128, 6, K], f32)
            nc.sync.dma_start(out=attnkT_sb,
                              in_=attnkT[:, :].rearrange("(c p) k -> p c k", p=128))
            attn2T_sb = constp.tile([128, 6, K], f32)
            nc.sync.dma_start(out=attn2T_sb,
                              in_=attn2T[:, :].rearrange("(c p) k -> p c k", p=128))

            # Preheat: have PE consume each constant once so no later matmul
            # needs >1 semaphore wait (walrus allows only 1 on fp32 matmuls).
            scr = scrp.tile([1, 1], f32)
            nc.tensor.matmul(scr, ones[:1, :], ones[:1, :], start=True, stop=True)
            nc.tensor.matmul(scr, attnkT_sb[:1, 0, :1], attnkT_sb[:1, 0, :1],
                             start=True, stop=True)
            nc.tensor.matmul(scr, attn2T_sb[:1, 0, :1], attn2T_sb[:1, 0, :1],
                             start=True, stop=True)
            nc.tensor.matmul(scr, prflat_sb[:1, :1], prflat_sb[:1, :1],
                             start=True, stop=True)

            # token sums, transposed: sumsT[d % 128, d // 128, b] = sum_n x[b,n,d]
            sumsT = psap.tile([128, 6, B], f32)

            # --- stage 1: stream x, copy to out[:, 8:, :], accumulate token sums
            for ci in range(B // C):
                b0 = ci * C
                xa = xap.tile([128, C, EMBED_DIM], f32)
                xb = xbp.tile([128, C, EMBED_DIM], f32)
                nc.gpsimd.dma_start(
                    out=xa,
                    in_=x[b0:b0 + C, 0:NA, :].rearrange("b n d -> n b d"))
                nc.gpsimd.dma_start(
                    out=xb[:NB],
                    in_=x[b0:b0 + C, NA:N_TOK, :].rearrange("b n d -> n b d"))
                nc.gpsimd.dma_start(
                    out=out[b0:b0 + C, LENGTH:LENGTH + NA, :].rearrange(
                        "b n d -> n b d"),
                    in_=xa)
                nc.gpsimd.dma_start(
                    out=out[b0:b0 + C, LENGTH + NA:LENGTH + N_TOK, :].rearrange(
                        "b n d -> n b d"),
                    in_=xb[:NB])
                for j in range(C):
                    b = b0 + j
                    for dc in range(6):
                        nc.tensor.matmul(sumsT[:, dc, b:b + 1],
                                         xa[:, j, ts(dc, 128)], ones[:128, :],
                                         start=True, stop=False)
                        nc.tensor.matmul(sumsT[:, dc, b:b + 1],
                                         xb[:NB, j, ts(dc, 128)], ones[:NB, :],
                                         start=False, stop=True)

            # --- stage 2: numer/norm2, aq ---
            meansT = miscp.tile([128, 6, B], f32)
            nc.vector.tensor_copy(meansT, sumsT)
            sqT = miscp.tile([128, 6, B], f32)
            nc.vector.tensor_mul(sqT, meansT, meansT)

            pn = pstp.tile([K, B], f32)
            pq = pstp.tile([K, B], f32)
            for j in range(6):
                nc.tensor.matmul(pn, attnkT_sb[:, j, :], meansT[:, j, :],
                                 start=(j == 0), stop=(j == 5))
            for j in range(6):
                nc.tensor.matmul(pq, attn2T_sb[:, j, :], sqT[:, j, :],
                                 start=(j == 0), stop=(j == 5))

            denom = miscp.tile([K, B], f32)
            nc.scalar.sqrt(denom, pq)
            nc.vector.tensor_scalar_max(denom, denom, 1e-12)
            recip = miscp.tile([K, B], f32)
            nc.vector.reciprocal(recip, denom)
            aqT = miscp.tile([K, B], f32)
            nc.vector.tensor_mul(aqT, pn, recip)

            # --- stage 3: P_ = aq @ prflat, write out[:, :8, :] ---
            p_sb = miscp.tile([B, PF], f32)
            for h in range(PF // 384):
                pp = ppp.tile([B, 384], f32)
                nc.tensor.matmul(pp, aqT, prflat_sb[:, ts(h, 384)],
                                 start=True, stop=True)
                nc.vector.tensor_copy(p_sb[:, ts(h, 384)], pp)
            nc.gpsimd.dma_start(
                out=out[:, 0:LENGTH, :],
                in_=p_sb.rearrange("p (l d) -> p l d", l=LENGTH))

    nc.finalize()
    return nc


def _host_prep(prompt, attention, prompt_key, task_count):
    K = (int(task_count) + 1) * TOP_K
    pk = np.asarray(prompt_key[:K], dtype=np.float32)
    att = np.asarray(attention[:K], dtype=np.float32)
    pr = np.asarray(prompt[:K], dtype=np.float32)
    nrm = np.sqrt(np.sum(pk * pk, axis=1, keepdims=True, dtype=np.float32))
    nK = pk / np.maximum(nrm, np.float32(1e-12))
    attnkT = np.ascontiguousarray((att * nK).T)
    attn2T = np.ascontiguousarray((att * att).T)
    prflat = np.ascontiguousarray(pr.reshape(K, PF))
    return K, attnkT, attn2T, prflat


def kernel(x_embed, prompt, attention, prompt_key, iseval, task_count,
           _want_trace=False, **_trace_kwargs):
    from concourse.bass_utils import run_bass_kernel_spmd

    x_embed = np.asarray(x_embed, dtype=np.float32)
    assert x_embed.shape == (B_FULL, N_TOK, EMBED_DIM)
    K, attnkT, attn2T, prflat = _host_prep(prompt, attention, prompt_key,
                                           task_count)

    if K not in _PROGRAMS:
        _PROGRAMS[K] = _build_program(K)
    nc = _PROGRAMS[K]

    in_maps = []
    for i in range(N_CORES):
        in_maps.append({
            "x": np.ascontiguousarray(x_embed[i * B:(i + 1) * B]),
            "prflat": prflat,
            "attnkT": attnkT,
            "attn2T": attn2T,
        })
    res = run_bass_kernel_spmd(nc, in_maps, core_ids=list(range(N_CORES)),
                               trace=_want_trace, **_trace_kwargs)
    full = np.concatenate([res.results[i]["out"] for i in range(N_CORES)],
                          axis=0)
    if _want_trace:
        return full, res
    return full
